# revision 13
# baseline (speedup 1.0000x reference)
"""Fused multi-head attention for Trainium2, SPMD across 8 NeuronCores.

Problem: B=2, T=2048, D=1024, H=16 heads (DH=64), fp32.
Returns (out[B,T,D], att_w[B,H,T,T]) matching the reference nn.Module.

Sharding: core c -> (batch b = c//4, head-group g = c%4).  Each core owns
4 heads of one batch: Wq/Wk/Wv column slice [:, 256g:256g+256], Wo row
slice [256g:256g+256, :].  Output projection partials are summed on host
(tensor-parallel reduction); att_w shards concatenate.

Per-core device pipeline (all heavy matmuls in float32r):
  1. QT/KT/VT = W.T @ X.T  (feature-major [256,2048]); V re-transposed
     to token-major via PE transposes for the AV matmul.
  2. Transposed scores  sT[k,q] = k.q/8 + mask_k*(-1e9)  via an augmented
     contraction row (mask lives in row 64 of the K operand, ones in row
     64 of the Q operand).  exp on ACT -> u[k,q] (unnormalized).
  3. AV matmul with a ones-column appended to V: rows 0-63 = unnormalized
     out.T, row 64 = softmax denominator (free).
  4. ln(denom) written into row 65 of the Q operand; natural-orientation
     scores s[q,k] = q.k/8 + mask - ln(denom) in ONE matmul (row 65 of the
     K operand = -1), so exp directly yields normalized att_w tiles.
  5. outT rows normalized by 1/denom (PE broadcast + DVE mul), then the
     output projection A @ Wo accumulated over the 4 heads.
"""

import sys

if "/opt/trn_rl_repo" not in sys.path:
    sys.path.insert(0, "/opt/trn_rl_repo")

import numpy as np

import concourse.bacc as bacc
import concourse.tile as tile
import concourse.mybir as mybir
from concourse.bass_utils import run_bass_kernel_spmd
from concourse.masks import make_identity

F32 = mybir.dt.float32
F32R = mybir.dt.float32r
AF = mybir.ActivationFunctionType
ALU = mybir.AluOpType

B, T, D, H = 2, 2048, 1024, 16
DH = D // H
NCORES = 8
HPC = 4            # heads per core
FPC = HPC * DH     # 256 features per core
KT = D // 128      # 8 k-tiles for projections
TT = T // 128      # 16 token tiles

TRACE = False
LAST_RESULTS = None


def build_nc():
    nc = bacc.Bacc("TRN2", target_bir_lowering=False, debug=False,
                   num_devices=NCORES)

    # ---- DRAM I/O (per-core shard, host-prepared layouts) ----
    xq = nc.dram_tensor("xq", [KT, 128, T], F32, kind="ExternalInput")   # X_q^T
    xk = nc.dram_tensor("xk", [KT, 128, T], F32, kind="ExternalInput")
    xv = nc.dram_tensor("xv", [KT, 128, T], F32, kind="ExternalInput")
    wq = nc.dram_tensor("wq", [128, KT, FPC], F32, kind="ExternalInput")  # /8 host
    wk = nc.dram_tensor("wk", [128, KT, FPC], F32, kind="ExternalInput")
    wv = nc.dram_tensor("wv", [128, KT, FPC], F32, kind="ExternalInput")
    wo = nc.dram_tensor("wo", [128, 2, D], F32, kind="ExternalInput")
    bq = nc.dram_tensor("bq", [128, 2], F32, kind="ExternalInput")        # /8 host
    bk = nc.dram_tensor("bk", [128, 2], F32, kind="ExternalInput")
    maskneg = nc.dram_tensor("maskneg", [1, T], F32, kind="ExternalInput")
    onesT = nc.dram_tensor("onesT", [1, T], F32, kind="ExternalInput")
    onesV = nc.dram_tensor("onesV", [128, TT], F32, kind="ExternalInput")
    att4 = nc.dram_tensor("att4", [HPC, T, T], F32, kind="ExternalOutput")
    DEBUG = bool(__import__("os").environ.get("KDEBUG"))
    if DEBUG:
        qdump = nc.dram_tensor("qdump", [HPC, 128, T], F32, kind="ExternalOutput")
        kdump = nc.dram_tensor("kdump", [HPC, 128, T], F32, kind="ExternalOutput")
        vdump = nc.dram_tensor("vdump", [HPC, 128, TT * (DH + 1)], F32,
                               kind="ExternalOutput")
    yp = nc.dram_tensor("yp", [T, D], F32, kind="ExternalOutput")

    with tile.TileContext(nc) as tc:
        # ---------- persistent SBUF ----------
        with tc.tile_pool(name="persist", bufs=1) as pp:
            qaug = [pp.tile([128, T], F32R, tag=f"qaug{h}", name=f"qaug{h}")
                    for h in range(HPC)]
            kaug = [pp.tile([128, T], F32R, tag=f"kaug{h}", name=f"kaug{h}")
                    for h in range(HPC)]
            # vaug[h]: [tok128, kt16, 64 feat + ones]
            vaug = [pp.tile([128, TT, DH + 1], F32R, tag=f"vaug{h}",
                             name=f"vaug{h}") for h in range(HPC)]
            at01 = pp.tile([128, T], F32R, tag="at01")   # A^T heads 0,1
            at23 = pp.tile([128, T], F32R, tag="at23")   # A^T heads 2,3
            wo_sb = pp.tile([128, 2, D], F32R, tag="wo_sb")
            ident = pp.tile([128, 128], F32, tag="ident")
            ones_row = pp.tile([1, DH], F32, tag="ones_row")
            one_one = pp.tile([1, 1], F32, tag="one_one")

            make_identity(nc, ident[:])
            nc.vector.memset(ones_row[:], 1.0)
            nc.vector.memset(one_one[:], 1.0)
            nc.gpsimd.dma_start(out=wo_sb[:], in_=wo[:])

            # aug row 64: q = 1, k = mask*(-1e9)  (k.q/8 + mask in one matmul)
            # (memset cannot produce f32r -> DMA-cast the constants from host)
            for h in range(HPC):
                nc.gpsimd.dma_start(out=qaug[h][64:65, :], in_=onesT[:])
                nc.gpsimd.dma_start(out=kaug[h][64:65, :], in_=maskneg[:])
                nc.gpsimd.dma_start(out=vaug[h][:, :, DH:DH + 1],
                                    in_=onesV[:, :, None])

            # ---------- phase 1: projections ----------
            with tc.tile_pool(name="p1w", bufs=1) as p1w, \
                 tc.tile_pool(name="p1x", bufs=2) as p1x, \
                 tc.tile_pool(name="p1ps", bufs=1, space="PSUM") as p1ps:
                w_sb = {}
                for nm, wt in (("q", wq), ("k", wk), ("v", wv)):
                    w_sb[nm] = p1w.tile([128, KT, FPC], F32R, tag=f"w{nm}",
                                        name=f"w{nm}")
                    nc.gpsimd.dma_start(out=w_sb[nm][:], in_=wt[:])
                b_sb = {}
                for nm, bt in (("q", bq), ("k", bk)):
                    b_sb[nm] = p1w.tile([128, 2], F32, tag=f"b{nm}", name=f"b{nm}")
                    nc.sync.dma_start(out=b_sb[nm][:], in_=bt[:])

                def project(xdram, wname, evac):
                    """PT[f,t] = sum_K W[K,f] * X^T[K,t]; evac(ps, m, n)."""
                    ps = [p1ps.tile([128, 512], F32, tag=f"ps{m}_{n}",
                                    name=f"ps{m}_{n}")
                          for m in range(2) for n in range(4)]
                    for kt in range(KT):
                        xt = p1x.tile([128, T], F32R, tag="xstream")
                        nc.gpsimd.dma_start(out=xt[:], in_=xdram[kt])
                        for m in range(2):
                            for n in range(4):
                                nc.tensor.matmul(
                                    ps[m * 4 + n][:],
                                    w_sb[wname][:, kt, m * 128:(m + 1) * 128],
                                    xt[:, n * 512:(n + 1) * 512],
                                    start=(kt == 0), stop=(kt == KT - 1))
                    for m in range(2):
                        for n in range(4):
                            evac(ps[m * 4 + n], m, n)

                def evac_qk(dst, bias):
                    def evac(ps, m, n):
                        for hh in range(2):
                            h = 2 * m + hh
                            nc.vector.tensor_scalar(
                                out=dst[h][0:DH, n * 512:(n + 1) * 512],
                                in0=ps[hh * DH:(hh + 1) * DH, :],
                                scalar1=bias[hh * DH:(hh + 1) * DH, m:m + 1],
                                scalar2=None, op0=ALU.add)
                    return evac

                project(xq, "q", evac_qk(qaug, b_sb["q"]))
                project(xk, "k", evac_qk(kaug, b_sb["k"]))

                # V: evac VT to staging, then PE-transpose into vaug
                vt_st = p1w.tile([128, 2, T], F32, tag="vt_st")

                def evac_v(ps, m, n):
                    nc.vector.tensor_copy(
                        vt_st[:, m, n * 512:(n + 1) * 512], ps[:])

                project(xv, "v", evac_v)

                for m in range(2):
                    for tb in range(TT):
                        # reuse projection psum slots (all 8 banks are taken)
                        pt = p1ps.tile([128, 128], F32, tag=f"ps0_{tb % 4}")
                        nc.tensor.transpose(
                            pt[:], vt_st[:, m, tb * 128:(tb + 1) * 128],
                            ident[:])
                        for hh in range(2):
                            h = 2 * m + hh
                            nc.vector.tensor_copy(
                                vaug[h][:, tb, 0:DH],
                                pt[:, hh * DH:(hh + 1) * DH])

            if DEBUG:
                for h in range(HPC):
                    nc.sync.dma_start(out=qdump[h], in_=qaug[h][:].bitcast(F32))
                    nc.sync.dma_start(out=kdump[h], in_=kaug[h][:].bitcast(F32))
                    nc.sync.dma_start(
                        out=vdump[h],
                        in_=vaug[h][:].bitcast(F32).rearrange("p a b -> p (a b)"))

            # ---------- phase 2: attention ----------
            with tc.tile_pool(name="p2u", bufs=3) as p2u, \
                 tc.tile_pool(name="p2att", bufs=3) as p2att, \
                 tc.tile_pool(name="p2sm", bufs=2) as p2sm, \
                 tc.tile_pool(name="psS", bufs=2, space="PSUM") as psS_p, \
                 tc.tile_pool(name="psAV", bufs=1, space="PSUM") as psAV_p, \
                 tc.tile_pool(name="psN", bufs=2, space="PSUM") as psN_p:
                for h in range(HPC):
                    for q2 in range(2):     # 1024-wide query blocks
                        q0 = q2 * 1024
                        pav = psAV_p.tile([DH + 1, 1024], F32, tag="pav")
                        for kt in range(TT):
                            pss = psS_p.tile([128, 1024], F32, tag="pss")
                            for j in range(2):
                                nc.tensor.matmul(
                                    pss[:, j * 512:(j + 1) * 512],
                                    kaug[h][0:65, kt * 128:(kt + 1) * 128],
                                    qaug[h][0:65, q0 + j * 512:q0 + (j + 1) * 512],
                                    start=True, stop=True)
                            u = p2u.tile([128, 1024], F32R, tag="u")
                            nc.scalar.activation(u[:], pss[:], AF.Exp)
                            for j in range(2):
                                nc.tensor.matmul(
                                    pav[:, j * 512:(j + 1) * 512],
                                    vaug[h][:, kt, :],
                                    u[:, j * 512:(j + 1) * 512],
                                    start=(kt == 0), stop=(kt == TT - 1))
                        # denominators: ln(d) row + 1/d to normalize outT
                        at_dst = at01 if h < 2 else at23
                        prow = (h % 2) * DH
                        lnd = p2sm.tile([1, 1024], F32, tag="lnd")
                        nc.scalar.activation(lnd[:], pav[DH:DH + 1, :], AF.Ln)
                        for j in range(2):
                            c0, c1 = q0 + j * 512, q0 + (j + 1) * 512
                            rec = p2sm.tile([1, 512], F32, tag="rec")
                            nc.vector.reciprocal(
                                rec[:], pav[DH:DH + 1, j * 512:(j + 1) * 512])
                            prc = psN_p.tile([DH, 512], F32, tag="psn")
                            nc.tensor.matmul(prc[:], ones_row[:], rec[:],
                                             start=True, stop=True)
                            rbc = p2sm.tile([DH, 512], F32, tag="rbc")
                            nc.vector.tensor_copy(rbc[:], prc[:])
                            nc.vector.tensor_mul(
                                at_dst[prow:prow + DH, c0:c1],
                                pav[0:DH, j * 512:(j + 1) * 512],
                                rbc[:])
                        # natural orientation: exp(s - ln d) = normalized att_w
                        for qs in range(8):
                            row0 = q0 + qs * 128
                            # -ln(d) into partition layout [128,1] via K=1 matmul
                            pcol = psN_p.tile([128, 1], F32, tag="psn")
                            nc.tensor.matmul(
                                pcol[:], lnd[0:1, qs * 128:(qs + 1) * 128],
                                one_one[:], start=True, stop=True)
                            nlnd = p2sm.tile([128, 1], F32, tag="nlnd")
                            nc.vector.tensor_scalar(
                                out=nlnd[:], in0=pcol[:], scalar1=-1.0,
                                scalar2=None, op0=ALU.mult)
                            asb = p2att.tile([128, T], F32, tag="asb")
                            for kn in range(4):
                                psn = psN_p.tile([128, 512], F32, tag="psn")
                                nc.tensor.matmul(
                                    psn[:],
                                    qaug[h][0:65, row0:row0 + 128],
                                    kaug[h][0:65, kn * 512:(kn + 1) * 512],
                                    start=True, stop=True)
                                nc.scalar.activation(
                                    asb[:, kn * 512:(kn + 1) * 512], psn[:],
                                    AF.Exp, bias=nlnd[:])
                            nc.sync.dma_start(out=att4[h, row0:row0 + 128, :],
                                              in_=asb[:])

            # ---------- phase 3: output projection ----------
            with tc.tile_pool(name="p3y", bufs=2) as p3y, \
                 tc.tile_pool(name="psY", bufs=4, space="PSUM") as psY_p:
                for ts_ in range(TT):
                    ysb = p3y.tile([128, D], F32, tag="ysb")
                    for n in range(2):
                        py = psY_p.tile([128, 512], F32, tag="py")
                        nc.tensor.matmul(
                            py[:], at01[:, ts_ * 128:(ts_ + 1) * 128],
                            wo_sb[:, 0, n * 512:(n + 1) * 512],
                            start=True, stop=False)
                        nc.tensor.matmul(
                            py[:], at23[:, ts_ * 128:(ts_ + 1) * 128],
                            wo_sb[:, 1, n * 512:(n + 1) * 512],
                            start=False, stop=True)
                        nc.scalar.copy(ysb[:, n * 512:(n + 1) * 512], py[:])
                    nc.sync.dma_start(out=yp[ts_ * 128:(ts_ + 1) * 128, :],
                                      in_=ysb[:])

    nc.compile()
    return nc


_NC = None


def kernel(query=None, key_in=None, value=None, mask=None, Wq=None, bq=None,
           Wk=None, bk=None, Wv=None, bv=None, Wo=None, bo=None, key=None,
           **_ignored):
    global _NC, LAST_RESULTS
    if key_in is None:
        key_in = key
    query = np.ascontiguousarray(np.asarray(query, dtype=np.float32))
    key_in = np.ascontiguousarray(np.asarray(key_in, dtype=np.float32))
    value = np.ascontiguousarray(np.asarray(value, dtype=np.float32))
    mask_np = np.asarray(mask)
    Wq = np.asarray(Wq, dtype=np.float32)
    Wk = np.asarray(Wk, dtype=np.float32)
    Wv = np.asarray(Wv, dtype=np.float32)
    Wo = np.asarray(Wo, dtype=np.float32)
    bq_np = np.asarray(bq, dtype=np.float32)
    bk_np = np.asarray(bk, dtype=np.float32)
    bv_np = np.asarray(bv, dtype=np.float32)
    bo_np = np.asarray(bo, dtype=np.float32)

    if _NC is None:
        _NC = build_nc()

    scale = 1.0 / np.sqrt(np.float32(DH))
    xT = {}
    for b in range(B):
        xT[("q", b)] = np.ascontiguousarray(query[b].T).reshape(KT, 128, T)
        xT[("k", b)] = np.ascontiguousarray(key_in[b].T).reshape(KT, 128, T)
        xT[("v", b)] = np.ascontiguousarray(value[b].T).reshape(KT, 128, T)

    in_maps = []
    for c in range(NCORES):
        b, g = c // HPC, c % HPC
        F = slice(g * FPC, (g + 1) * FPC)
        in_maps.append({
            "xq": xT[("q", b)],
            "xk": xT[("k", b)],
            "xv": xT[("v", b)],
            "wq": np.ascontiguousarray(
                (Wq[:, F] * scale).reshape(KT, 128, FPC).transpose(1, 0, 2)),
            "wk": np.ascontiguousarray(
                Wk[:, F].reshape(KT, 128, FPC).transpose(1, 0, 2)),
            "wv": np.ascontiguousarray(
                Wv[:, F].reshape(KT, 128, FPC).transpose(1, 0, 2)),
            "wo": np.ascontiguousarray(
                Wo[F, :].reshape(2, 128, D).transpose(1, 0, 2)),
            "bq": np.ascontiguousarray((bq_np[F] * scale).reshape(2, 128).T),
            "bk": np.ascontiguousarray(bk_np[F].reshape(2, 128).T),
            "maskneg": (mask_np[b, 0, 0, :].astype(np.float32)
                        * np.float32(-1e9)).reshape(1, T),
            "onesT": np.ones((1, T), np.float32),
            "onesV": np.ones((128, TT), np.float32),
        })

    res = run_bass_kernel_spmd(_NC, in_maps, list(range(NCORES)), trace=TRACE)
    LAST_RESULTS = res

    att_w = np.empty((B, H, T, T), dtype=np.float32)
    out = np.zeros((B, T, D), dtype=np.float32)
    for c in range(NCORES):
        b, g = c // HPC, c % HPC
        att_w[b, g * HPC:(g + 1) * HPC] = res.results[c]["att4"]
        out[b] += res.results[c]["yp"]
    out += (bv_np @ Wo + bo_np)[None, None, :]
    return out, att_w
